# revision 14
# baseline (speedup 1.0000x reference)
"""Trainium2 Bass kernel for GraphSToV (gnn_message_passing).

Computes out[b,i,j,c,k] = relu(dist[b,i,j,c] * (s[b,i,k] + t[b,j,k] + bias[k]))
where s = x @ W[:F], t = x @ W[F:].

Sharding: data-parallel over batch B=16 across 8 cores (2 batches/core);
weights replicated. Output (403MB fp32) dominates traffic -> memory-bound:
per-core roofline ~50.3MB / ~358GB/s ~= 141us.

Per-core device program (Tile framework):
  per batch b (inputs x^T, all weights and a ones col packed in ONE DMA'd
  tile `hx` so matmuls carry at most one sync-wait - PE matmuls only
  support a single one):
    [s | t+bias | 1] = matmul(lhsT=[x^T;1], rhs=[W1,0 | W2,bias | 0,1])
    t_rows = [t+bias | 1] flattened onto one partition (SBUF->SBUF DMA)
             so each row is a base-0 matmul operand
    per neighbor j (128):
      pp   = ones_i (x) t_rows[j]     (K=1 broadcast matmul -> PSUM)
      pair = pp + s                   (VectorE)
      out[:, j, c, :] = relu(pair * dist[:, j, c])   c=0,1,2
        (ScalarE activation w/ per-partition scale x2, VectorE
         tensor_scalar mult+max0 x1)
    output staged in [128, JT*3*128] tiles, DMA'd as JT-column groups
    (12KB contiguous per partition row -> descriptor-efficient).
"""

import sys

for _p in ("/opt/trn_rl_repo",):
    if _p not in sys.path:
        sys.path.insert(0, _p)

import numpy as np

import concourse.bacc as bacc
import concourse.bass as bass
import concourse.mybir as mybir
import concourse.tile as tile
from concourse.bass_utils import run_bass_kernel_spmd

FP32 = mybir.dt.float32

B, N, F, FILT, COOR = 16, 128, 64, 128, 3
NCORES = 8
BL = B // NCORES   # batches per core
JT = 8             # neighbor columns per output DMA group
WC = 2 * FILT + 1  # packed weight cols: [W1|0, W2|bias, 0|1]

TRACE = False          # set by test harness for profiling
LAST_RESULTS = None    # BassKernelResults of the last run


def build_nc(jt: int = JT) -> bass.Bass:
    nc = bacc.Bacc(None, target_bir_lowering=False)
    hx = nc.dram_tensor("hx", [BL, F + 1, N + WC], FP32, kind="ExternalInput")
    dist = nc.dram_tensor("dist", [BL, N, N, COOR], FP32, kind="ExternalInput")
    out = nc.dram_tensor("out", [BL, N, N, COOR, FILT], FP32, kind="ExternalOutput")

    relu = mybir.ActivationFunctionType.Relu
    mult = mybir.AluOpType.mult
    amax = mybir.AluOpType.max

    with tile.TileContext(nc) as tc:
        with (
            tc.tile_pool(name="hxp", bufs=2) as hxp,
            tc.tile_pool(name="dlp", bufs=2) as dlp,
            tc.tile_pool(name="stp", bufs=2) as stp,
            tc.tile_pool(name="trp", bufs=2) as trp,
            tc.tile_pool(name="st_psum", bufs=2, space="PSUM") as st_psum,
            tc.tile_pool(name="pair_psA", bufs=3, space="PSUM") as pair_psA,
            tc.tile_pool(name="pair_psB", bufs=3, space="PSUM") as pair_psB,
            tc.tile_pool(name="pairp", bufs=4) as pairp,
            tc.tile_pool(name="outp", bufs=3) as outp,
        ):
            for b in range(BL):
                hxt = hxp.tile([F + 1, N + WC], FP32, tag="hx")
                nc.sync.dma_start(out=hxt, in_=hx[b])
                dlt = dlp.tile([N, N, COOR], FP32, tag="dl")
                nc.sync.dma_start(out=dlt, in_=dist[b])

                # [s | t+bias | ones] in one matmul: [128, 257]
                pst = st_psum.tile([N, WC], FP32, tag="pst")
                nc.tensor.matmul(
                    pst, hxt[:, 0:N], hxt[:, N : N + WC], start=True, stop=True
                )
                st = stp.tile([N, WC], FP32, tag="st")
                nc.scalar.copy(st, pst)
                # [t+bias | ones] flattened to one partition: row j at
                # [j*(FILT+1)], its trailing 1.0 at [j*(FILT+1)+FILT]
                trows = trp.tile([1, N, FILT + 1], FP32, tag="trows")
                nc.sync.dma_start(out=trows, in_=st[:, FILT:WC])

                pair_ps = pair_psA if b == 0 else pair_psB
                for jg in range(N // jt):
                    ot = outp.tile([N, jt, COOR, FILT], FP32, tag="ot")
                    for jj in range(jt):
                        j = jg * jt + jj
                        # pp = ones_i (x) (t+bias)[j]
                        pp = pair_ps.tile([N, FILT], FP32, tag=f"pp{b}")
                        nc.tensor.matmul(
                            pp,
                            trows[0:1, :, FILT],
                            trows[0:1, j, 0:FILT],
                            start=True, stop=True,
                        )
                        # pair = pp + s  (VectorE; PSUM-capable)
                        pair = pairp.tile([N, FILT], FP32, tag="pair")
                        nc.vector.tensor_add(pair, pp, st[:, 0:FILT])
                        # 3 scaled-relus: ScalarE c0,c1; VectorE c2
                        nc.scalar.activation(
                            ot[:, jj, 0, :], pair, relu, scale=dlt[:, j, 0:1]
                        )
                        nc.scalar.activation(
                            ot[:, jj, 1, :], pair, relu, scale=dlt[:, j, 1:2]
                        )
                        nc.vector.tensor_scalar(
                            ot[:, jj, 2, :], pair, dlt[:, j, 2:3], 0.0, mult, amax
                        )
                    nc.sync.dma_start(
                        out=out[b, :, jg * jt : (jg + 1) * jt, :, :], in_=ot
                    )
    nc.compile()
    return nc


_NC = None


def _get_nc() -> bass.Bass:
    global _NC
    if _NC is None:
        _NC = build_nc()
    return _NC


def kernel(scalar_features, distances, w_sv, b_sv):
    global LAST_RESULTS
    x = np.ascontiguousarray(np.asarray(scalar_features, dtype=np.float32))
    d = np.ascontiguousarray(np.asarray(distances, dtype=np.float32))
    w = np.asarray(w_sv, dtype=np.float32)
    bias = np.asarray(b_sv, dtype=np.float32)
    assert x.shape == (B, N, F) and d.shape == (B, N, N, COOR)

    xs = x.reshape(NCORES, BL, N, F)
    dsh = d.reshape(NCORES, BL, N, N, COOR)

    # packed weights [F+1, 2*FILT+1]: [W1|0, W2|bias, 0|1]
    w_cat = np.zeros((F + 1, WC), dtype=np.float32)
    w_cat[:F, :FILT] = w[:F]
    w_cat[:F, FILT : 2 * FILT] = w[F:]
    w_cat[F, FILT : 2 * FILT] = bias
    w_cat[F, 2 * FILT] = 1.0

    in_maps = []
    for c in range(NCORES):
        hx = np.empty((BL, F + 1, N + WC), np.float32)
        hx[:, :F, :N] = xs[c].transpose(0, 2, 1)
        hx[:, F, :N] = 1.0
        hx[:, :, N:] = w_cat
        in_maps.append(
            {
                "hx": np.ascontiguousarray(hx),
                "dist": np.ascontiguousarray(dsh[c]),
            }
        )

    nc = _get_nc()
    res = run_bass_kernel_spmd(nc, in_maps, list(range(NCORES)), trace=TRACE)
    LAST_RESULTS = res
    outs = np.concatenate([res.results[i]["out"] for i in range(NCORES)], axis=0)
    return outs.reshape(B, N, N, COOR, FILT)
